# revision 1
# baseline (speedup 1.0000x reference)
"""Deformable conv (DCNv2-style) TRN2 Bass kernel.

Problem: x[8,64,128,128] f32; offset conv (27ch 3x3) -> (dy,dx,mask) per 9 taps;
bilinear sampling of x at tap positions + offsets; modulated; 3x3 conv via
per-tap 1x1 matmuls.

Strategy (per core, data-parallel over batch, 8 cores):
  - om conv: 9 shifted matmuls on zero-padded XP (f32r, full speed) -> PSUM
    [27,512] chunks -> redistribute to OMT [h-part, j, w] via DMA.
  - stage2 (DVE/ACT on [128h, 9t, 128w]): floor/clip/hat-weights/mask/sigmoid
    -> W4A/W4B (bf16 corner-weight pairs), idx (uint16, element units into X3).
  - X3: interleaved row-pair layout X3[c, r, 2j]=x[r,j], [.., 2j+1]=x[r+1,j]
    (bf16); partitions 64-127 hold X3 shifted by 2 elements (the x0+1 corners).
    One indirect_copy (inner=2) per (tap, 512-pos chunk) fetches 4 corners
    for all 64 channels x 2 column-halves.
  - weights replicated across partitions via SBUF->SBUF DMA (Wlin staging ->
    per-chunk broadcast).
  - P = G * Wrep (DVE bf16); einsum: per chunk accumulate 9 taps x 2 slots
    matmuls (lhsT = w_conv tap slices stacked x2) into PSUM [64,512] -> out.

Gather stream order: position s = 512*m + col*16 + p  (IDENTITY: s = 512m + i).
"""
import numpy as np
import ml_dtypes

from concourse.bacc import Bacc
from concourse import mybir, tile
from concourse.bass_utils import run_bass_kernel_spmd

np_bf16 = np.dtype(ml_dtypes.bfloat16)
f32 = mybir.dt.float32
f32r = mybir.dt.float32r
bf16 = mybir.dt.bfloat16
u16 = mybir.dt.uint16
i32 = mybir.dt.int32

B, C, H, W = 8, 64, 128, 128
HW = H * W          # 16384
T = 9               # taps
NJ = 27             # offset-conv channels
NCHUNK = 32         # 512-position chunks
CHUNK = 512
AF = mybir.ActivationFunctionType
ALU = mybir.AluOpType

_CACHE = {}


def _host_consts():
    # CYK[h, t] = h + ky(t) - 1 ; CXW[h, t, w] = w + kx(t) - 1 (h-independent)
    ky = np.arange(T) // 3
    kx = np.arange(T) % 3
    cyk = (np.arange(128)[:, None] + ky[None, :] - 1).astype(np.float32)
    cxw = np.broadcast_to(
        (np.arange(128)[None, :] + kx[:, None] - 1).astype(np.float32),
        (128, T, 128)).copy()
    # note: cxw[h, t, w] must be w + kx - 1 for every h
    cxw = np.broadcast_to(
        (np.arange(128)[None, :] + kx[:, None] - 1)[None, :, :], (128, T, 128)
    ).astype(np.float32).copy()
    return cyk, cxw


def build_nc(num_devices=8, debug_taps=None):
    """Build the per-core SPMD kernel. debug_taps: if set, also emit debug
    tensors (idx, W4) for verification."""
    nc = Bacc("TRN2", target_bir_lowering=False, debug=False,
              num_devices=num_devices)

    x_in = nc.dram_tensor("x_in", [C, HW], f32, kind="ExternalInput")
    woffT_in = nc.dram_tensor("woffT_in", [C, T * NJ], f32, kind="ExternalInput")
    boff_in = nc.dram_tensor("boff_in", [NJ, 1], f32, kind="ExternalInput")
    wk2_in = nc.dram_tensor("wk2_in", [128, T * C], bf16, kind="ExternalInput")
    out_dram = nc.dram_tensor("out", [C, HW], f32, kind="ExternalOutput")

    cyk_np, cxw_np = _host_consts()
    cyk_const = nc.inline_tensor(cyk_np, name="cyk_const")
    cxw_const = nc.inline_tensor(cxw_np.reshape(128, T * 128), name="cxw_const")

    dbg = {}

    with tile.TileContext(nc) as tc:
        with tc.tile_pool(name="main", bufs=1) as mp, \
             tc.tile_pool(name="dram", bufs=1, space="DRAM") as drp, \
             tc.tile_pool(name="dbuf", bufs=2) as dbp:
            # ---------- persistent tiles ----------
            X3 = mp.tile([128, HW * 2], bf16, tag="X3")          # 64 KiB
            W4C = mp.tile([128, 2, T, 128, 2], bf16, tag="W4C")  # [h,half,t,w,slot]
            IDXF = mp.tile([128, T, 8, 16], u16, tag="IDXF")     # [h,t,a,b] (a=w//16,b=w%16)
            IDXT = mp.tile([128, T, NCHUNK, 32], u16, tag="IDXT")
            WLIN = drp.tile([NCHUNK, 2, T, CHUNK, 2], bf16, tag="WLIN")
            CYK = mp.tile([128, T], f32, tag="CYK")
            CXW = mp.tile([128, T, 128], f32, tag="CXW")
            WOFFT = mp.tile([C, T, NJ], f32r, tag="WOFFT")
            WK2 = mp.tile([128, T, C], bf16, tag="WK2")
            BOFF = mp.tile([NJ, 1], f32, tag="BOFF")

            nc.sync.dma_start(CYK[:], cyk_const.ap())
            nc.sync.dma_start(CXW[:].rearrange("p a b -> p (a b)"), cxw_const.ap())
            nc.sync.dma_start(WK2[:].rearrange("p a b -> p (a b)"), wk2_in.ap())
            nc.sync.dma_start(BOFF[:], boff_in.ap())

            # ================= Phase A: pad + X3 + om conv =================
            midcm = tc.tile_pool(name="mid", bufs=1)
            midp = midcm.__enter__()
            OMT = midp.tile([128, NJ, 128], f32, tag="OMT")      # [h, j, w]
            with tc.tile_pool(name="early", bufs=1) as ep, \
                 tc.tile_pool(name="ompsum", bufs=2, space="PSUM") as opp:
                XP = ep.tile([C, 130 * 130], f32r, tag="XP")
                WOFFS = ep.tile([C, T * NJ], f32, tag="WOFFS")
                nc.sync.dma_start(WOFFS[:], woffT_in.ap())
                nc.vector.tensor_copy(out=WOFFT[:].rearrange("p a b -> p (a b)"),
                                      in_=WOFFS[:])

                nc.vector.memset(XP[:].bitcast(f32), 0.0)
                XP3 = XP[:].rearrange("p (r c2) -> p r c2", c2=130)
                nc.gpsimd.dma_start(out=XP3[:, 1:129, 1:129], in_=x_in.ap())

                # X3 A-half: X3[c, r*256 + 2j + s] = x[c, r+s, j]
                X3A = X3[0:64, :].rearrange("p (r j s) -> p r j s", j=128, s=2)
                nc.scalar.copy(out=X3A[:, :, :, 0], in_=XP3[0:64, 1:129, 1:129].bitcast(f32))
                nc.vector.tensor_copy(out=X3A[:, :, :, 1], in_=XP3[0:64, 2:130, 1:129].bitcast(f32))
                # B-half: shift by 2 elements (cross-partition copy via DMA)
                nc.sync.dma_start(out=X3[64:128, 0:2 * HW - 2], in_=X3[0:64, 2:2 * HW])
                nc.vector.memset(X3[64:128, 2 * HW - 2:2 * HW], 0.0)

                # om conv: per 512-pos chunk, 9 accumulating f32r matmuls
                for m in range(NCHUNK):
                    ps = opp.tile([NJ, CHUNK], f32, tag="omps")
                    for t9 in range(T):
                        ty, tx = divmod(t9, 3)
                        rhs = XP3[0:64, 4 * m + ty: 4 * m + ty + 4, tx: tx + 128]
                        nc.tensor.matmul(ps[:], lhsT=WOFFT[:, t9, :], rhs=rhs,
                                         start=(t9 == 0), stop=(t9 == T - 1))
                    omp = dbp.tile([NJ, CHUNK], f32, tag="omp")
                    nc.scalar.activation(out=omp[:], in_=ps[:], func=AF.Identity,
                                         bias=BOFF[:])
                    # redistribute: OMT[4m+hp, j, :] = omp[j, hp*128:(hp+1)*128]
                    ompv = omp[:].rearrange("j (hp w) -> j hp w", w=128)
                    for hp in range(4):
                        nc.sync.dma_start(
                            out=OMT[4 * m + hp: 4 * m + hp + 1, :, :],
                            in_=ompv[:, hp, :])
            # ================= stage 2: weights + idx =================
            with tc.tile_pool(name="s2", bufs=1) as sp:
                OMTv = OMT[:]
                DY = OMTv[:, 0:18, :].rearrange("p (k s) w -> p k s w", s=2)[:, :, 0, :]
                DX = OMTv[:, 0:18, :].rearrange("p (k s) w -> p k s w", s=2)[:, :, 1, :]
                MS = OMTv[:, 18:27, :]

                sh = [128, T, 128]
                YS = sp.tile(sh, f32, tag="YS")
                XS = sp.tile(sh, f32, tag="XS")
                Y0C = sp.tile(sh, f32, tag="Y0C")
                X0C = sp.tile(sh, f32, tag="X0C")
                TMPI = sp.tile(sh, i32, tag="TMPI")
                TY = sp.tile(sh, f32, tag="TY")
                TX = sp.tile(sh, f32, tag="TX")
                WYA = sp.tile(sh, f32, tag="WYA")
                WYB = sp.tile(sh, f32, tag="WYB")
                WXA = sp.tile(sh, f32, tag="WXA")
                WXB = sp.tile(sh, f32, tag="WXB")
                MSK = sp.tile(sh, f32, tag="MSK")
                TMP = sp.tile(sh, f32, tag="TMP")
                TMP2 = sp.tile(sh, f32, tag="TMP2")

                CYKb = CYK[:].unsqueeze(2).broadcast_to(sh)

                # ys/xs
                nc.vector.tensor_tensor(out=YS[:], in0=DY, in1=CYKb, op=ALU.add)
                nc.vector.tensor_tensor(out=XS[:], in0=DX, in1=CXW[:], op=ALU.add)
                # floor via round(x - 0.5) (cast round-to-nearest-even)
                for SRC, DSTF in ((YS, Y0C), (XS, X0C)):
                    nc.vector.tensor_scalar(out=TMP[:], in0=SRC[:], scalar1=0.5,
                                            scalar2=None, op0=ALU.subtract)
                    nc.vector.tensor_copy(out=TMPI[:], in_=TMP[:])
                    nc.vector.tensor_copy(out=DSTF[:], in_=TMPI[:])
                    # clip to [0, 127]
                    nc.vector.tensor_scalar(out=DSTF[:], in0=DSTF[:], scalar1=0.0,
                                            scalar2=127.0, op0=ALU.max, op1=ALU.min)
                # t = s - clip ; weights
                nc.vector.tensor_tensor(out=TY[:], in0=YS[:], in1=Y0C[:], op=ALU.subtract)
                nc.vector.tensor_tensor(out=TX[:], in0=XS[:], in1=X0C[:], op=ALU.subtract)
                # wA = relu(1 - |t|), wBr = relu(t)
                nc.scalar.activation(out=TMP[:], in_=TY[:], func=AF.Abs)
                nc.scalar.activation(out=WYA[:], in_=TMP[:], func=AF.Relu, scale=-1.0, bias=1.0)
                nc.scalar.activation(out=WYB[:], in_=TY[:], func=AF.Relu)
                nc.scalar.activation(out=TMP2[:], in_=TX[:], func=AF.Abs)
                nc.scalar.activation(out=WXA[:], in_=TMP2[:], func=AF.Relu, scale=-1.0, bias=1.0)
                nc.scalar.activation(out=WXB[:], in_=TX[:], func=AF.Relu)
                # upper-boundary masks: wyB *= (ys < 127); wxB *= (xs < 127)
                nc.vector.tensor_scalar(out=TMP[:], in0=YS[:], scalar1=127.0,
                                        scalar2=None, op0=ALU.is_lt)
                nc.vector.tensor_tensor(out=WYB[:], in0=WYB[:], in1=TMP[:], op=ALU.mult)
                nc.vector.tensor_scalar(out=TMP2[:], in0=XS[:], scalar1=127.0,
                                        scalar2=None, op0=ALU.is_lt)
                nc.vector.tensor_tensor(out=WXB[:], in0=WXB[:], in1=TMP2[:], op=ALU.mult)
                # mask; fold into wx
                nc.scalar.activation(out=MSK[:], in_=MS, func=AF.Sigmoid)
                nc.vector.tensor_tensor(out=WXA[:], in0=WXA[:], in1=MSK[:], op=ALU.mult)
                nc.vector.tensor_tensor(out=WXB[:], in0=WXB[:], in1=MSK[:], op=ALU.mult)
                # products -> W4A/W4B (bf16, interleaved)
                nc.vector.tensor_tensor(out=W4C[:, 0, :, :, 0], in0=WYA[:], in1=WXA[:], op=ALU.mult)
                nc.vector.tensor_tensor(out=W4C[:, 0, :, :, 1], in0=WYB[:], in1=WXA[:], op=ALU.mult)
                nc.vector.tensor_tensor(out=W4C[:, 1, :, :, 0], in0=WYA[:], in1=WXB[:], op=ALU.mult)
                nc.vector.tensor_tensor(out=W4C[:, 1, :, :, 1], in0=WYB[:], in1=WXB[:], op=ALU.mult)
                # idx (element units into X3): 2*(y0c*128 + x0c), written in
                # IDXF layout [h, t, a, b] with value for w = 16a + b
                nc.vector.scalar_tensor_tensor(
                    out=TMP[:], in0=Y0C[:], scalar=128.0, in1=X0C[:],
                    op0=ALU.mult, op1=ALU.add)
                IDXF_w = IDXF[:].rearrange("p t a b2 -> p (t a b2)").rearrange(
                    "p (t a b2) -> p t a b2", t=T, a=8)
                # out enumeration must follow in (t, w) = (t, a, b)
                nc.vector.tensor_scalar(
                    out=IDXF_w, in0=TMP[:].rearrange("p t (a b2) -> p t a b2", a=8),
                    scalar1=2.0, scalar2=None, op0=ALU.mult)

                if debug_taps:
                    d_w4 = nc.dram_tensor("d_w4", [128, 2 * T * 128 * 2], bf16, kind="ExternalOutput")
                    d_idx = nc.dram_tensor("d_idx", [128, T * 8 * 16], u16, kind="ExternalOutput")
                    d_omt = nc.dram_tensor("d_omt", [128, NJ * 128], f32, kind="ExternalOutput")
                    nc.sync.dma_start(d_w4.ap(), W4C[:].rearrange("p a b c d -> p (a b c d)"))
                    nc.sync.dma_start(d_idx.ap(), IDXF[:].rearrange("p a b c -> p (a b c)"))
                    nc.sync.dma_start(d_omt.ap(), OMT[:].rearrange("p a b -> p (a b)"))

            # ---------- IDXT build: 16 DMAs + 1 replication ----------
            # IDXT[p, t, m, col] = idx(s = 512m + col*16 + p)
            #                    = IDXF[h = 4m + col//8, t, a = col%8, b = p]
            for p16 in range(16):
                for t9 in range(T):
                    # in: IDXF[:, t9, :, p16] [128h-part, 8a]; enumer (h, a)
                    # out: IDXT[p16, t9, m, col=(hl a)]; enumer (m, hl, a)=(h, a)
                    nc.sync.dma_start(
                        out=IDXT[p16:p16 + 1, t9, :, :].rearrange(
                            "o m (hl a) -> o (m hl) a", hl=4),
                        in_=IDXF[:, t9, :, p16])
            # replicate partitions 0-15 -> 16-127 (doubling)
            nc.sync.dma_start(out=IDXT[16:32], in_=IDXT[0:16])
            nc.sync.dma_start(out=IDXT[32:64], in_=IDXT[0:32])
            nc.sync.dma_start(out=IDXT[64:128], in_=IDXT[0:64])
            midcm.__exit__(None, None, None)

            # ---------- WLIN build: 8 DMAs (SBUF -> DRAM) ----------
            # WLIN[m, half, t, hl*128 + w, slot] = W4C[4m+hl, half, t, w, slot]
            WLINv = WLIN[:]
            for half in range(2):
                for hl in range(4):
                    nc.sync.dma_start(
                        out=WLINv[:, half, :, hl * 128:(hl + 1) * 128, :],
                        in_=W4C[hl:128:4, half, :, :, :])

            # ================= main loop =================
            with tc.tile_pool(name="gl", bufs=2) as gp, \
                 tc.tile_pool(name="ps2", bufs=1, space="PSUM") as pp2:
                X3v = X3[:].rearrange("p (n s) -> p n s", s=2)
                # absorb initial deps into gpsimd queue
                dd1 = mp.tile([128, 1], bf16, tag="dd1")
                dd2 = mp.tile([128, 1], u16, tag="dd2")
                nc.gpsimd.tensor_copy(out=dd1[:], in_=X3[:, 0:1])
                nc.gpsimd.tensor_copy(out=dd2[:], in_=IDXT[:, 0, 0, 0:1])

                for Q in range(4):
                    psums = [pp2.tile([C, CHUNK], f32, tag=f"eps{c8}",
                                      name=f"eps_{Q}_{c8}")
                             for c8 in range(8)]
                    outsb_list = []
                    for tap in range(T):
                        GQ = gp.tile([128, 8 * CHUNK, 2], bf16, tag="GQ")
                        WRQ = gp.tile([128, 8, CHUNK, 2], bf16, tag="WRQ")
                        dump_this = debug_taps and Q == 0 and tap == 0
                        for c8 in range(8):
                            m = 8 * Q + c8
                            nc.gpsimd.indirect_copy(
                                out=GQ[:, CHUNK * c8:CHUNK * (c8 + 1), :],
                                data=X3v, idxs=IDXT[:, tap, m, :],
                                i_know_ap_gather_is_preferred=True)
                            # Wrep: two half-broadcasts per chunk (from DRAM)
                            nc.sync.dma_start(
                                out=WRQ[0:64, c8, :, :],
                                in_=WLINv[m, 0, tap, :, :].unsqueeze(0)
                                    .broadcast_to([64, CHUNK, 2]))
                            nc.sync.dma_start(
                                out=WRQ[64:128, c8, :, :],
                                in_=WLINv[m, 1, tap, :, :].unsqueeze(0)
                                    .broadcast_to([64, CHUNK, 2]))
                        if dump_this:
                            d_gq = nc.dram_tensor("d_gq", [128, 8 * CHUNK * 2], bf16, kind="ExternalOutput")
                            d_wrq = nc.dram_tensor("d_wrq", [128, 8 * CHUNK * 2], bf16, kind="ExternalOutput")
                            nc.sync.dma_start(d_gq.ap(), GQ[:].rearrange("p a b -> p (a b)"))
                            nc.sync.dma_start(d_wrq.ap(), WRQ[:].rearrange("p a b c -> p (a b c)"))
                        nc.vector.tensor_tensor(
                            out=GQ[:].rearrange("p a b -> p (a b)"),
                            in0=GQ[:].rearrange("p a b -> p (a b)"),
                            in1=WRQ[:].rearrange("p a b c -> p (a b c)"),
                            op=ALU.mult)
                        PQv = GQ[:].rearrange("p (m s) two -> p m s two", m=8)
                        for c8 in range(8):
                            for slot in range(2):
                                nc.tensor.matmul(
                                    psums[c8][:], lhsT=WK2[:, tap, :],
                                    rhs=PQv[:, c8, :, slot],
                                    start=(tap == 0 and slot == 0),
                                    stop=(tap == T - 1 and slot == 1))
                    for c8 in range(8):
                        m = 8 * Q + c8
                        osb = dbp.tile([C, CHUNK], f32, tag="osb")
                        nc.scalar.copy(out=osb[:], in_=psums[c8][:])
                        nc.sync.dma_start(
                            out=out_dram.ap()[:, m * CHUNK:(m + 1) * CHUNK],
                            in_=osb[:])

    nc.compile()
    return nc


def _prep_weights(w_offset, b_offset, w_conv):
    w_offset = np.asarray(w_offset, dtype=np.float32)
    w_conv = np.asarray(w_conv, dtype=np.float32)
    b_offset = np.asarray(b_offset, dtype=np.float32)
    # woffT[c, t*27 + j] = w_offset[j, c, ty, tx]
    woffT = w_offset.transpose(2, 3, 1, 0).reshape(T, C, NJ)  # [t, c, j]
    woffT = woffT.transpose(1, 0, 2).reshape(C, T * NJ).copy()
    boff = b_offset.reshape(NJ, 1).copy()
    # wk2[q, t*64 + o] = w_conv[o, q%64, ty, tx]
    wkt = w_conv.transpose(2, 3, 1, 0).reshape(T, C, C)       # [t, c, o]
    wk2 = np.concatenate([wkt, wkt], axis=1)                   # [t, 128, o]
    wk2 = wk2.transpose(1, 0, 2).reshape(128, T * C).astype(np_bf16).copy()
    return woffT, boff, wk2


def kernel(x, w_offset, b_offset, w_conv):
    x = np.asarray(x, dtype=np.float32)
    woffT, boff, wk2 = _prep_weights(w_offset, b_offset, w_conv)
    if "nc" not in _CACHE:
        _CACHE["nc"] = build_nc(num_devices=B)
    nc = _CACHE["nc"]
    in_maps = []
    for b in range(B):
        in_maps.append({
            "x_in": np.ascontiguousarray(x[b].reshape(C, HW)),
            "woffT_in": woffT,
            "boff_in": boff,
            "wk2_in": wk2,
        })
    res = run_bass_kernel_spmd(nc, in_maps, core_ids=list(range(B)))
    out = np.stack([res.results[b]["out"].reshape(C, H, W) for b in range(B)])
    return out.astype(np.float32)



# revision 30
# speedup vs baseline: 1.1568x; 1.1568x over previous
"""Deformable conv (DCNv2-style) TRN2 Bass kernel — DMA-restructured.

Problem: x[8,64,128,128] f32; offset conv (27ch 3x3) -> (dy,dx,mask) per 9 taps;
bilinear sampling of x at tap positions + offsets; modulated; 3x3 conv via
per-tap 1x1 matmuls.  Data-parallel over batch, 1 sample per core.

Per-core pipeline:
  A) om conv: 9 shifted f32r matmuls per 512-pos chunk on zero-padded XP
     -> OM_ALL [27, 16384]; ONE transpose DMA -> OMT [h, j, w].
  B) stage2 (DVE/ACT on [128h, 9t, 128w]): floor/clip/hat-weights/sigmoid
     -> W4C (bf16 corner-weight pairs) and IDXF (u16 element idx into X3).
  C) X3: interleaved row-pair layout X3[c, 2*(128r+j)+s] = x[c, r+s, j] (bf16);
     partitions 64-127 hold X3 shifted by 2 elements (x0+1 corners).
  D) IDXT: 16+3 DMAs -> idx wrapped per 16 partitions [p, t, m, col].
     WLIN staging: W4C -> DRAM as [t][half, m, hl, w, slot] (18 small DMAs).
  E) main loop over 4 Qs x 9 taps: ONE indirect_copy (4096 idx, d=2),
     ONE 2 MB 128-partition broadcast DMA for weights, ONE DVE mult,
     16 accumulating matmuls -> PSUM [64,512] x 8 chunks; per Q: 8 ACT
     copies -> OSBQ, ONE 1 MB DMA to out.

Gather stream order: position s = 512*m + col*16 + p.
"""
import numpy as np
import ml_dtypes

from concourse.bacc import Bacc
from concourse import mybir, tile
from concourse.bass_utils import run_bass_kernel_spmd

np_bf16 = np.dtype(ml_dtypes.bfloat16)
f32 = mybir.dt.float32
f32r = mybir.dt.float32r
bf16 = mybir.dt.bfloat16
u16 = mybir.dt.uint16
i32 = mybir.dt.int32

B, C, H, W = 8, 64, 128, 128
HW = H * W          # 16384
T = 9               # taps
NJ = 27             # offset-conv channels
NCHUNK = 32         # 512-position chunks
CHUNK = 512
NQ = 4              # chunk groups
QC = NCHUNK // NQ   # 8 chunks per group
AF = mybir.ActivationFunctionType
ALU = mybir.AluOpType

_CACHE = {}


def _host_consts():
    # CYK[h, t] = h + ky(t) - 1 ; CXW[h, t, w] = w + kx(t) - 1 (h-independent)
    ky = np.arange(T) // 3
    kx = np.arange(T) % 3
    cyk = (np.arange(128)[:, None] + ky[None, :] - 1).astype(np.float32)
    cxw = np.broadcast_to(
        (np.arange(128)[None, :] + kx[:, None] - 1)[None, :, :], (128, T, 128)
    ).astype(np.float32).copy()
    return cyk, cxw


def build_nc(num_devices=8):
    import os
    variant = os.environ.get("KVARIANT", "full")
    v_wrq2 = "wrq2" in variant or "oldall" in variant
    v_omtold = "omtold" in variant or "oldall" in variant
    v_idxold = "idxold" in variant or "oldall" in variant
    nc = Bacc("TRN2", target_bir_lowering=False, debug=False,
              num_devices=num_devices)
    kdebug = os.environ.get("KDEBUG") == "1"

    x_in = nc.dram_tensor("x_in", [C, HW], f32, kind="ExternalInput")
    woffT_in = nc.dram_tensor("woffT_in", [C, T * NJ], f32, kind="ExternalInput")
    boff_in = nc.dram_tensor("boff_in", [NJ, 1], f32, kind="ExternalInput")
    wk2_in = nc.dram_tensor("wk2_in", [128, T * C], bf16, kind="ExternalInput")
    out_dram = nc.dram_tensor("out", [C, HW], f32, kind="ExternalOutput")

    cyk_np, cxw_np = _host_consts()
    cyk_const = nc.inline_tensor(cyk_np, name="cyk_const")
    cxw_const = nc.inline_tensor(cxw_np.reshape(128, T * 128), name="cxw_const")

    with tile.TileContext(nc) as tc:
        with tc.tile_pool(name="main", bufs=1) as mp, \
             tc.tile_pool(name="dram", bufs=1, space="DRAM") as drp:
            # ---------- persistent tiles ----------
            X3 = mp.tile([128, HW * 2 + 16], bf16, tag="X3")     # 64 KiB + pad
            W4C = mp.tile([128, 2, T, 128, 2], bf16, tag="W4C")  # [h,half,t,w,slot]
            IDXF = mp.tile([128, T, 8, 16], u16, tag="IDXF")     # [h,t,a,b] (a=w//16,b=w%16)
            IDXT = mp.tile([128, T, NCHUNK, 32], u16, tag="IDXT")
            # WLIN row (t*2+half) = [m, hl, w, slot] flat; DRAM staging for
            # broadcast reads (flat 2-D: >2-D DRAM tile slicing miscompiles)
            WLIN = drp.tile([T * 2, NCHUNK * 4 * 128 * 2], bf16, tag="WLIN")
            CYK = mp.tile([128, T], f32, tag="CYK")
            CXW = mp.tile([128, T, 128], f32, tag="CXW")
            WOFFT = mp.tile([C, T, NJ], bf16, tag="WOFFT")
            WK2 = mp.tile([128, T, C], bf16, tag="WK2")
            BOFF = mp.tile([NJ, 1], f32, tag="BOFF")

            nc.sync.dma_start(CYK[:], cyk_const.ap())
            nc.sync.dma_start(CXW[:].rearrange("p a b -> p (a b)"), cxw_const.ap())
            nc.sync.dma_start(WK2[:].rearrange("p a b -> p (a b)"), wk2_in.ap())
            nc.sync.dma_start(BOFF[:], boff_in.ap())

            # ================= Phase A: pad + X3 + om conv =================
            midcm = tc.tile_pool(name="mid", bufs=1)
            midp = midcm.__enter__()
            OMT = midp.tile([128, NJ, 128], f32, tag="OMT")      # [h, j, w]
            with tc.tile_pool(name="early", bufs=1) as ep, \
                 tc.tile_pool(name="omdb", bufs=2) as odb, \
                 tc.tile_pool(name="ompsum", bufs=2, space="PSUM") as opp:
                XP = ep.tile([C, 130 * 130], bf16, tag="XP")
                WOFFS = ep.tile([C, T * NJ], f32, tag="WOFFS")
                nc.sync.dma_start(WOFFS[:], woffT_in.ap())
                nc.vector.tensor_copy(out=WOFFT[:].rearrange("p a b -> p (a b)"),
                                      in_=WOFFS[:])

                # zero only the halo ring of XP (interior is overwritten by x)
                XP3 = XP[:].rearrange("p (r c2) -> p r c2", c2=130)
                nc.vector.memset(XP[:, 0:130], 0.0)                  # top row
                nc.vector.memset(XP[:, 129 * 130:130 * 130], 0.0)    # bottom row
                nc.vector.memset(XP3[:, 1:129, 0], 0.0)              # left col
                nc.vector.memset(XP3[:, 1:129, 129], 0.0)            # right col
                nc.gpsimd.dma_start(out=XP3[:, 1:129, 1:129], in_=x_in.ap())

                # X3 A-half: X3[c, r*256 + 2j + s] = x[c, r+s, j]
                X3A = X3[0:64, 0:HW * 2].rearrange("p (r j s) -> p r j s", j=128, s=2)
                nc.scalar.copy(out=X3A[:, :, :, 0], in_=XP3[0:64, 1:129, 1:129])
                nc.vector.tensor_copy(out=X3A[:, :, :, 1], in_=XP3[0:64, 2:130, 1:129])
                nc.vector.memset(X3[0:64, HW * 2:], 0.0)
                # B-half: shift by 2 elements (cross-partition copy via DMA)
                nc.sync.dma_start(out=X3[64:128, 0:2 * HW - 2], in_=X3[0:64, 2:2 * HW])
                nc.vector.memset(X3[64:128, 2 * HW - 2:], 0.0)

                # om conv: per 512-pos chunk, 9 accumulating f32r matmuls;
                # stage 4 chunks into omp4 -> DRAM scratch -> transpose load
                OMD = drp.tile([NJ, HW], f32, tag="OMD")   # DRAM [j, (h w)]
                for g in range(NCHUNK // 4):
                    omp4 = odb.tile([NJ, 4, CHUNK], f32, tag="omp4")
                    for mi in range(4):
                        m = 4 * g + mi
                        ps = opp.tile([NJ, CHUNK], f32, tag="omps")
                        for t9 in range(T):
                            ty, tx = divmod(t9, 3)
                            rhs = XP3[0:64, 4 * m + ty: 4 * m + ty + 4, tx: tx + 128]
                            nc.tensor.matmul(ps[:], lhsT=WOFFT[:, t9, :], rhs=rhs,
                                             start=(t9 == 0), stop=(t9 == T - 1))
                        nc.scalar.activation(out=omp4[:, mi, :], in_=ps[:],
                                             func=AF.Identity, bias=BOFF[:])
                    if v_omtold:
                        ompv = omp4[:].rearrange("j m4 (hp w) -> j (m4 hp) w", w=128)
                        for hp in range(16):
                            nc.sync.dma_start(
                                out=OMT[16 * g + hp:16 * g + hp + 1, :, :],
                                in_=ompv[:, hp, :])
                    else:
                        nc.sync.dma_start(
                            out=OMD[:][:, 2048 * g:2048 * (g + 1)],
                            in_=omp4[:].rearrange("j m4 p -> j (m4 p)"))
                if not v_omtold:
                    # ONE transpose load: OMT[h, j, w] <- OMD[j, (h w)]
                    nc.sync.dma_start(
                        out=OMT[:],
                        in_=OMD[:].rearrange("j (h w) -> h j w", w=128))
            # ================= stage 2: weights + idx =================
            with tc.tile_pool(name="s2", bufs=1) as sp:
                OMTv = OMT[:]
                DY = OMTv[:, 0:18, :].rearrange("p (k s) w -> p k s w", s=2)[:, :, 0, :]
                DX = OMTv[:, 0:18, :].rearrange("p (k s) w -> p k s w", s=2)[:, :, 1, :]
                MS = OMTv[:, 18:27, :]

                sh = [128, T, 128]
                YS = sp.tile(sh, f32, tag="YS")
                XS = sp.tile(sh, f32, tag="XS")
                Y0C = sp.tile(sh, f32, tag="Y0C")
                X0C = sp.tile(sh, f32, tag="X0C")
                TMPI = sp.tile(sh, i32, tag="TMPI")
                TY = sp.tile(sh, f32, tag="TY")
                TX = sp.tile(sh, f32, tag="TX")
                WYA = sp.tile(sh, f32, tag="WYA")
                WYB = sp.tile(sh, f32, tag="WYB")
                WXA = sp.tile(sh, f32, tag="WXA")
                WXB = sp.tile(sh, f32, tag="WXB")
                MSK = sp.tile(sh, f32, tag="MSK")
                TMP = sp.tile(sh, f32, tag="TMP")
                TMP2 = sp.tile(sh, f32, tag="TMP2")

                CYKb = CYK[:].unsqueeze(2).broadcast_to(sh)

                # ys/xs
                nc.vector.tensor_tensor(out=YS[:], in0=DY, in1=CYKb, op=ALU.add)
                nc.vector.tensor_tensor(out=XS[:], in0=DX, in1=CXW[:], op=ALU.add)
                # floor via round(x - 0.5) (cast round-to-nearest-even)
                for SRC, DSTF in ((YS, Y0C), (XS, X0C)):
                    nc.vector.tensor_scalar(out=TMP[:], in0=SRC[:], scalar1=0.5,
                                            scalar2=None, op0=ALU.subtract)
                    nc.vector.tensor_copy(out=TMPI[:], in_=TMP[:])
                    nc.vector.tensor_copy(out=DSTF[:], in_=TMPI[:])
                    # clip to [0, 127]
                    nc.vector.tensor_scalar(out=DSTF[:], in0=DSTF[:], scalar1=0.0,
                                            scalar2=127.0, op0=ALU.max, op1=ALU.min)
                # t = s - clip ; weights
                nc.vector.tensor_tensor(out=TY[:], in0=YS[:], in1=Y0C[:], op=ALU.subtract)
                nc.vector.tensor_tensor(out=TX[:], in0=XS[:], in1=X0C[:], op=ALU.subtract)
                # wA = relu(1 - |t|), wBr = relu(t)
                nc.scalar.activation(out=TMP[:], in_=TY[:], func=AF.Abs)
                nc.scalar.activation(out=WYA[:], in_=TMP[:], func=AF.Relu, scale=-1.0, bias=1.0)
                nc.scalar.activation(out=WYB[:], in_=TY[:], func=AF.Relu)
                nc.scalar.activation(out=TMP2[:], in_=TX[:], func=AF.Abs)
                nc.scalar.activation(out=WXA[:], in_=TMP2[:], func=AF.Relu, scale=-1.0, bias=1.0)
                nc.scalar.activation(out=WXB[:], in_=TX[:], func=AF.Relu)
                # upper-boundary masks: wyB *= (ys < 127); wxB *= (xs < 127)
                nc.vector.tensor_scalar(out=TMP[:], in0=YS[:], scalar1=127.0,
                                        scalar2=None, op0=ALU.is_lt)
                nc.vector.tensor_tensor(out=WYB[:], in0=WYB[:], in1=TMP[:], op=ALU.mult)
                nc.vector.tensor_scalar(out=TMP2[:], in0=XS[:], scalar1=127.0,
                                        scalar2=None, op0=ALU.is_lt)
                nc.vector.tensor_tensor(out=WXB[:], in0=WXB[:], in1=TMP2[:], op=ALU.mult)
                # mask; fold into wx
                nc.scalar.activation(out=MSK[:], in_=MS, func=AF.Sigmoid)
                nc.vector.tensor_tensor(out=WXA[:], in0=WXA[:], in1=MSK[:], op=ALU.mult)
                nc.vector.tensor_tensor(out=WXB[:], in0=WXB[:], in1=MSK[:], op=ALU.mult)
                # products -> W4C (bf16, interleaved)
                nc.vector.tensor_tensor(out=W4C[:, 0, :, :, 0], in0=WYA[:], in1=WXA[:], op=ALU.mult)
                nc.vector.tensor_tensor(out=W4C[:, 0, :, :, 1], in0=WYB[:], in1=WXA[:], op=ALU.mult)
                nc.vector.tensor_tensor(out=W4C[:, 1, :, :, 0], in0=WYA[:], in1=WXB[:], op=ALU.mult)
                nc.vector.tensor_tensor(out=W4C[:, 1, :, :, 1], in0=WYB[:], in1=WXB[:], op=ALU.mult)
                # idx (element units into X3): 2*(y0c*128 + x0c), written in
                # IDXF layout [h, t, a, b] with value for w = 16a + b
                nc.vector.scalar_tensor_tensor(
                    out=TMP[:], in0=Y0C[:], scalar=128.0, in1=X0C[:],
                    op0=ALU.mult, op1=ALU.add)
                IDXF_w = IDXF[:].rearrange("p t a b2 -> p (t a b2)").rearrange(
                    "p (t a b2) -> p t a b2", t=T, a=8)
                nc.vector.tensor_scalar(
                    out=IDXF_w, in0=TMP[:].rearrange("p t (a b2) -> p t a b2", a=8),
                    scalar1=2.0, scalar2=None, op0=ALU.mult)

            # ---------- IDXT build: DRAM bounce + 16 loads + 3 replications ----
            # IDXT[p, t, m, col] = idx(s = 512m + col*16 + p)
            #                    = IDXF[h = 4m + col//8, t, a = col%8, b = p]
            if v_idxold:
                for p16 in range(16):
                    for t9 in range(T):
                        nc.sync.dma_start(
                            out=IDXT[p16:p16 + 1, t9, :, :].rearrange(
                                "o m (hl a) -> o (m hl) a", hl=4),
                            in_=IDXF[:, t9, :, p16])
            else:
                IDXD = drp.tile([T, NCHUNK * 4 * 8 * 16], u16, tag="IDXD")
                IDXDv = IDXD[:]
                for t9 in range(T):
                    nc.sync.dma_start(
                        out=IDXDv[t9].rearrange("(h f) -> h f", h=128),
                        in_=IDXF[:, t9, :, :].rearrange("p a b -> p (a b)"))
                IDXD5 = IDXDv.rearrange("t (m hl a b) -> t m hl a b",
                                        hl=4, a=8, b=16)
                for p16 in range(16):
                    nc.sync.dma_start(
                        out=IDXT[p16:p16 + 1, :, :, :].rearrange(
                            "o t m (hl a) -> o (t m hl a)", hl=4),
                        in_=IDXD5[:, :, :, :, p16].rearrange(
                            "t m hl a -> (t m hl a)").unsqueeze(0))
            # replicate partitions 0-15 -> 16-127 (doubling)
            nc.sync.dma_start(out=IDXT[16:32], in_=IDXT[0:16])
            nc.sync.dma_start(out=IDXT[32:64], in_=IDXT[0:32])
            nc.sync.dma_start(out=IDXT[64:128], in_=IDXT[0:64])
            midcm.__exit__(None, None, None)

            if kdebug:
                d_omt = nc.dram_tensor("d_omt", [128, NJ * 128], f32, kind="ExternalOutput")
                nc.sync.dma_start(d_omt.ap(), OMT[:].rearrange("p a b -> p (a b)"))
                d_w4 = nc.dram_tensor("d_w4", [128, 2 * T * 128 * 2], bf16, kind="ExternalOutput")
                nc.sync.dma_start(d_w4.ap(), W4C[:].rearrange("p a b c d -> p (a b c d)"))
                d_idxt = nc.dram_tensor("d_idxt", [128, T * NCHUNK * 32], u16, kind="ExternalOutput")
                nc.sync.dma_start(d_idxt.ap(), IDXT[:].rearrange("p a b c -> p (a b c)"))
                d_x3 = nc.dram_tensor("d_x3", [128, HW * 2], bf16, kind="ExternalOutput")
                nc.sync.dma_start(d_x3.ap(), X3[:, 0:HW * 2])

            # ---------- WLIN build: 18 small DMAs (SBUF -> DRAM) ----------
            # WLIN[t*2+half] = flat [m, hl, w, slot] = W4C[h=(m hl), half, t, w, s]
            WLINv = WLIN[:]
            for t9 in range(T):
                for half in range(2):
                    nc.sync.dma_start(
                        out=WLINv[t9 * 2 + half].rearrange("(h f) -> h f", h=128),
                        in_=W4C[:, half, t9, :, :].rearrange("p a b -> p (a b)"))

            if kdebug:
                d_wlin = nc.dram_tensor("d_wlin", [T * 2, NCHUNK * 4 * 128 * 2], bf16, kind="ExternalOutput")
                nc.sync.dma_start(d_wlin.ap(), WLINv)

            # ================= main loop =================
            with tc.tile_pool(name="gl", bufs=2) as gp, \
                 tc.tile_pool(name="ps2", bufs=1, space="PSUM") as pp2:
                X3v = X3[:, 0:HW * 2].rearrange("p (n s) -> p n s", s=2)
                # absorb initial deps into gpsimd queue
                dd1 = mp.tile([128, 1], bf16, tag="dd1")
                dd2 = mp.tile([128, 1], u16, tag="dd2")
                nc.gpsimd.tensor_copy(out=dd1[:], in_=X3[:, 0:1])
                nc.gpsimd.tensor_copy(out=dd2[:], in_=IDXT[:, 0, 0, 0:1])

                for Q in range(NQ):
                    psums = [pp2.tile([C, CHUNK], f32, tag=f"eps{c8}",
                                      name=f"eps_{Q}_{c8}")
                             for c8 in range(QC)]
                    for tap in range(T):
                        GQ = gp.tile([128, QC * CHUNK, 2], bf16, tag="GQ")
                        WRQ = gp.tile([128, QC, CHUNK, 2], bf16, tag="WRQ")
                        # gathers: per chunk (IC dst cap = 1024 elems/partition)
                        for c8 in range(QC):
                            nc.gpsimd.indirect_copy(
                                out=GQ[:, CHUNK * c8:CHUNK * (c8 + 1), :],
                                data=X3v,
                                idxs=IDXT[:, tap, QC * Q + c8, :],
                                i_know_ap_gather_is_preferred=True)
                        # 128-partition broadcast DMA (2 MB):
                        # WRQ[(half c), m, pos, slot] <- WLIN[tap, half, m(Q), :, :, :]
                        QB = QC * CHUNK * 2   # elems per (tap, half, Q)
                        for half in range(2):
                            nc.scalar.dma_start(
                                out=WRQ[64 * half:64 * (half + 1)],
                                in_=WLINv[tap * 2 + half,
                                          QB * Q:QB * (Q + 1)]
                                    .unsqueeze(0)
                                    .broadcast_to([64, QB])
                                    .rearrange("c (m pos s) -> c m pos s",
                                               m=QC, s=2))
                        if kdebug and Q == 0 and tap == 0:
                            d_gq = nc.dram_tensor("d_gq", [128, QC * CHUNK * 2], bf16, kind="ExternalOutput")
                            nc.sync.dma_start(d_gq.ap(), GQ[:].rearrange("p a b -> p (a b)"))
                            d_wrq = nc.dram_tensor("d_wrq", [128, QC * CHUNK * 2], bf16, kind="ExternalOutput")
                            nc.sync.dma_start(d_wrq.ap(), WRQ[:].rearrange("p a b c -> p (a b c)"))
                        nc.vector.tensor_tensor(
                            out=GQ[:].rearrange("p a b -> p (a b)"),
                            in0=GQ[:].rearrange("p a b -> p (a b)"),
                            in1=WRQ[:].rearrange("p a b c -> p (a b c)"),
                            op=ALU.mult)
                        PQv = GQ[:].rearrange("p (m s) two -> p m s two", m=QC)
                        for c8 in range(QC):
                            for slot in range(2):
                                nc.tensor.matmul(
                                    psums[c8][:], lhsT=WK2[:, tap, :],
                                    rhs=PQv[:, c8, :, slot],
                                    start=(tap == 0 and slot == 0),
                                    stop=(tap == T - 1 and slot == 1))
                    OSBQ = gp.tile([C, QC, CHUNK], f32, tag="OSBQ")
                    for c8 in range(QC):
                        nc.scalar.copy(out=OSBQ[:, c8, :], in_=psums[c8][:])
                    nc.sync.dma_start(
                        out=out_dram.ap()[:, Q * QC * CHUNK:(Q + 1) * QC * CHUNK],
                        in_=OSBQ[:].rearrange("p a b -> p (a b)"))

    nc.compile()
    return nc


def _prep_weights(w_offset, b_offset, w_conv):
    w_offset = np.asarray(w_offset, dtype=np.float32)
    w_conv = np.asarray(w_conv, dtype=np.float32)
    b_offset = np.asarray(b_offset, dtype=np.float32)
    # woffT[c, t*27 + j] = w_offset[j, c, ty, tx]
    woffT = w_offset.transpose(2, 3, 1, 0).reshape(T, C, NJ)  # [t, c, j]
    woffT = woffT.transpose(1, 0, 2).reshape(C, T * NJ).copy()
    boff = b_offset.reshape(NJ, 1).copy()
    # wk2[q, t*64 + o] = w_conv[o, q%64, ty, tx]
    wkt = w_conv.transpose(2, 3, 1, 0).reshape(T, C, C)       # [t, c, o]
    wk2 = np.concatenate([wkt, wkt], axis=1)                   # [t, 128, o]
    wk2 = wk2.transpose(1, 0, 2).reshape(128, T * C).astype(np_bf16).copy()
    return woffT, boff, wk2


def kernel(x, w_offset, b_offset, w_conv):
    x = np.asarray(x, dtype=np.float32)
    woffT, boff, wk2 = _prep_weights(w_offset, b_offset, w_conv)
    if "nc" not in _CACHE:
        _CACHE["nc"] = build_nc(num_devices=B)
    nc = _CACHE["nc"]
    in_maps = []
    for b in range(B):
        in_maps.append({
            "x_in": np.ascontiguousarray(x[b].reshape(C, HW)),
            "woffT_in": woffT,
            "boff_in": boff,
            "wk2_in": wk2,
        })
    res = run_bass_kernel_spmd(nc, in_maps, core_ids=list(range(B)))
    out = np.stack([res.results[b]["out"].reshape(C, H, W) for b in range(B)])
    return out.astype(np.float32)
